# revision 33
# baseline (speedup 1.0000x reference)
"""Trainium2 Bass kernel for nn_CustomCrossModalAttention (B=2, N=2048, D=768, H=12).

Sharding (8 cores, collective-free):
  - core c owns batch b = c//4 and query rows [512*(c%4), 512*(c%4)+512).
  - k'/v are computed REDUNDANTLY for the whole batch on each of its 4 cores
    (~45us extra PE) instead of exchanging shards: the AllGather pair cost far
    more than the replicated matmuls and serialized the whole pipeline.
  - Keys are column-PERMUTED per core so the core's own 512 rows come first
    (softmax sums over all keys, so key order is irrelevant); this makes the
    SPMD program uniform while the gate still reads "own" xv columns at a
    fixed offset 0.

Math folds (exact):
  - scores = (q@k^T)*scale + q@pos^T == scale * (q @ (LNk*g + lnk_b + pos/scale)^T)
  - LN_v gain/bias folded into wo/bo.
  - gate z-half folded through the output projection: gate = sigmoid(
      vis@gwv^T + attnout@(gwz@wo_eff)^T + gb + gwz@bo_a), removing the
    z -> zT transposes and the serialization on z.
  - All additive biases in this problem are structurally zero
    (setup_inputs uses jnp.zeros); nonzero biases are supported via
    ones-row matmuls compiled on demand (flags in the build cache key).

Dtypes (validated by numpy emulation to rel-err ~8e-3, same as the old
AllGather kernel): q path f32r end-to-end (q errors multiply the large q@pos
term in the exp argument, so bf16 there would cost ~2% at-error); k/v/gate/out
paths bf16; kT kept f32 (magnitude ~8 after the pos fold); exp/softmax in
bf16; all matmul accumulation f32 in PSUM.

Schedule: V-proj -> Q-proj -> K-proj -> per-head-pair [kT transpose block ->
attention heads 2s,2s+1] -> out-proj/gate/fuse/final-LN. Interleaving the kT
blocks with attention lets the Activation engine's softmax exp (~95us, the
2nd-busiest engine) start while PE is still projecting.
"""

import numpy as np
import ml_dtypes

B, N, D = 2, 2048, 768
H, DH = 12, 64
P = 128
CORES, GROUP = 8, 4
S = 512            # query rows per core
NCH = S // P       # 4 row chunks per core
MCH = N // P       # 16 key chunks
G6 = D // P        # 6
SCALE = DH ** -0.5
EPS = 1e-5

BF = ml_dtypes.bfloat16

_CACHE = {}

HALVES = [(0, 512), (512, D)]


def _build(has_qkv_bias, has_o_bias, has_g_bias, has_f_affine):
    from contextlib import ExitStack

    import concourse.bacc as bacc
    import concourse.mybir as mybir
    import concourse.tile as tile
    from concourse.masks import make_identity

    f32 = mybir.dt.float32
    f32r = mybir.dt.float32r
    bf16 = mybir.dt.bfloat16
    ALU = mybir.AluOpType
    ACTF = mybir.ActivationFunctionType

    nc = bacc.Bacc("TRN2", target_bir_lowering=False, num_devices=CORES)

    def din(name, shape, dt=bf16):
        return nc.dram_tensor(name, shape, dt, kind="ExternalInput")

    xqT = din("xqT", [D, S], f32r)        # own infrared rows, transposed
    xvT = din("xvT", [D, N], bf16)        # full-batch visible, transposed, key-permuted
    vis_nat = din("vis_nat", [S, D], f32)  # own visible rows, natural
    posTb = din("posTb", [D, N], bf16)    # pos/scale + lnk_b, transposed, permuted
    wqT = din("wqT", [D, D], f32r)
    wkvT = din("wkvT", [D, 2 * D], bf16)  # [wk.T | wv.T]
    woT = din("woT", [D, D], bf16)        # (wo * lnv_w).T
    gwvT = din("gwvT", [D, D], bf16)      # gate vis-half weights, transposed
    gwzT = din("gwzT", [D, D], bf16)      # (gwz @ wo_eff).T
    lnq_g = din("lnq_g", [P, G6], f32)
    lnq_b = din("lnq_b", [P, G6], f32)
    lnk_g = din("lnk_g", [P, G6], f32)
    bqkv = din("bqkv", [1, 3 * D], f32r)  # only read when has_qkv_bias
    bo_a = din("bo_a", [1, D], bf16)      # bo + wo@lnv_b
    gb_e = din("gb_e", [1, D], bf16)      # gate_b + gwz@bo_a
    lnf = din("lnf", [2, D], f32)
    out_rows = nc.dram_tensor("out_rows", [S, D], f32, kind="ExternalOutput")

    with tile.TileContext(nc) as tc, ExitStack() as ctx:
        const = ctx.enter_context(tc.tile_pool(name="const", bufs=1))
        persist = ctx.enter_context(tc.tile_pool(name="persist", bufs=1))

        ident_bf = const.tile([P, P], bf16)
        make_identity(nc, ident_bf)
        ident_f32 = const.tile([P, P], f32)
        make_identity(nc, ident_f32)
        ones_r_f = const.tile([1, P], f32)
        nc.vector.memset(ones_r_f, 1.0)
        ones_r = ones_r_f.bitcast(f32r)
        ones_b = const.tile([1, P], bf16)
        nc.vector.memset(ones_b, 1.0)
        eps_t = const.tile([P, 1], f32)
        nc.vector.memset(eps_t, EPS)

        # ---- persistent tiles ----
        xvT_sb = persist.tile([P, G6, N], bf16)
        kT_sb = persist.tile([P, G6, N], f32r)
        vaug_sb = persist.tile([P, MCH, H, DH + 1], bf16)
        qT_sb = persist.tile([P, G6, S], f32r)
        outT_sb = persist.tile([P, G6, S], bf16)
        lnq_g_sb = persist.tile([P, G6], f32)
        lnq_b_sb = persist.tile([P, G6], f32)
        lnk_g_sb = persist.tile([P, G6], f32)
        lnfw_sb = persist.tile([1, D], f32)
        lnfb_sb = persist.tile([1, D], f32)

        nc.vector.memset(vaug_sb[:, :, :, DH:DH + 1], 1.0)

        def ln_stats(y, pool):
            st = pool.tile([P, 2, 6], f32, tag="st")
            for i in range(2):
                nc.vector.bn_stats(out=st[:, i], in_=y[:, i * 384:(i + 1) * 384])
            mv = pool.tile([P, 2], f32, tag="mv")
            nc.vector.bn_aggr(out=mv, in_=st)
            rstd = pool.tile([P, 1], f32, tag="rstd")
            nc.scalar.activation(
                out=rstd, in_=mv[:, 1:2], func=ACTF.Sqrt, bias=eps_t, scale=1.0
            )
            nc.vector.reciprocal(out=rstd, in_=rstd)
            negmr = pool.tile([P, 1], f32, tag="negmr")
            nc.vector.tensor_scalar(
                out=negmr, in0=mv[:, 0:1], scalar1=rstd, scalar2=-1.0,
                op0=ALU.mult, op1=ALU.mult,
            )
            return negmr, rstd

        # ================= phase V + Q (scoped) =================
        with (
            tc.tile_pool(name="pvq", bufs=1) as pvq,
            tc.tile_pool(name="wrot", bufs=2) as wrot,
            tc.tile_pool(name="stat", bufs=6) as stat,
            tc.tile_pool(name="qn", bufs=1) as qn,
            tc.tile_pool(name="ps_p", bufs=3, space="PSUM") as ps_p,
            tc.tile_pool(name="ps_t", bufs=2, space="PSUM") as ps_t,
        ):
            # DMA issue order == need order: a 1-chunk xvT sliver and the
            # weight pieces unblock V-proj chunk 0 ASAP; xvT streams right
            # behind on the HWDGE queues; the q-path rides the slower SWDGE.
            nc.sync.dma_start(
                out=xvT_sb[:, :, 0:P],
                in_=xvT.rearrange("(s p) n -> p s n", p=P)[:, :, 0:P],
            )
            wv_sb = wrot.tile([P, G6, D], bf16, tag="wkv")
            for w0, w1 in ((0, 256), (256, 512), (512, D)):
                nc.scalar.dma_start(
                    out=wv_sb[:, :, w0:w1],
                    in_=wkvT.rearrange("(s p) o -> p s o", p=P)[:, :, D + w0:D + w1],
                )
            nc.sync.dma_start(
                out=xvT_sb[:, :, P:S],
                in_=xvT.rearrange("(s p) n -> p s n", p=P)[:, :, P:S],
            )
            for i, eng in ((1, nc.sync), (2, nc.scalar), (3, nc.sync)):
                eng.dma_start(
                    out=xvT_sb[:, :, i * S:(i + 1) * S],
                    in_=xvT.rearrange("(s p) n -> p s n", p=P)[:, :, i * S:(i + 1) * S],
                )
            bqkv_sb = None
            if has_qkv_bias:
                bqkv_sb = pvq.tile([1, 3 * D], f32r)
                nc.sync.dma_start(out=bqkv_sb, in_=bqkv.ap())

            def proj_tile(lhsT_sb, w_sb, w_off, c, halves=HALVES):
                py = ps_p.tile([P, D], f32, tag="py")
                for o0, o1 in halves:
                    for s in range(G6):
                        nc.tensor.matmul(
                            py[:, o0:o1],
                            lhsT_sb[:, s, c * P:(c + 1) * P],
                            w_sb[:, s, o0:o1],
                            start=(s == 0), stop=(not has_qkv_bias and s == G6 - 1),
                        )
                    if has_qkv_bias:
                        nc.tensor.matmul(
                            py[:, o0:o1], ones_r,
                            bqkv_sb[:, w_off + o0:w_off + o1],
                            start=False, stop=True,
                        )
                return py

            xqT_sb = pvq.tile([P, G6, S], f32r)
            nc.gpsimd.dma_start(
                out=xqT_sb, in_=xqT.rearrange("(s p) n -> p s n", p=P)
            )
            wq_sb = pvq.tile([P, G6, D], f32r)
            nc.gpsimd.dma_start(
                out=wq_sb, in_=wqT.rearrange("(s p) o -> p s o", p=P)
            )
            nc.gpsimd.dma_start(out=lnq_g_sb, in_=lnq_g.ap())
            nc.gpsimd.dma_start(out=lnq_b_sb, in_=lnq_b.ap())
            nc.gpsimd.dma_start(out=lnk_g_sb, in_=lnk_g.ap())
            nc.gpsimd.dma_start(out=lnfw_sb, in_=lnf.ap()[0:1, :])
            nc.gpsimd.dma_start(out=lnfb_sb, in_=lnf.ap()[1:2, :])

            def v_chunk(c, halves=HALVES):
                py = proj_tile(xvT_sb, wv_sb, 2 * D, c, halves)
                negmr, rstd = ln_stats(py, stat)
                nc.scalar.activation(
                    out=vaug_sb[:, c, :, 0:DH],
                    in_=py.rearrange("p (h d) -> p h d", h=H),
                    func=ACTF.Identity, bias=negmr, scale=rstd,
                )

            # ---- V (chunk 0 split finer so it starts on the first weight
            # piece), then Q; V 14,15 fill the q-proj -> q-transpose stall
            v_chunk(0, [(0, 256), (256, 512), (512, D)])
            for c in range(1, MCH - 2):
                v_chunk(c)

            qnats = []
            for c in range(NCH):
                py = proj_tile(xqT_sb, wq_sb, 0, c)
                negmr, rstd = ln_stats(py, stat)
                qnat = qn.tile([P, D], f32, tag=f"qnat{c}")
                nc.scalar.activation(
                    out=qnat, in_=py, func=ACTF.Identity, bias=negmr, scale=rstd
                )
                qnats.append(qnat)
            v_chunk(MCH - 2)
            v_chunk(MCH - 1)
            for s in range(G6):
                pt = ps_t.tile([P, NCH, P], f32, tag="pt")
                for c in range(NCH):
                    nc.tensor.transpose(
                        pt[:, c], qnats[c][:, s * P:(s + 1) * P], ident_f32
                    )
                nc.vector.tensor_scalar(
                    out=qT_sb[:, s, :],
                    in0=pt.rearrange("p c n -> p (c n)"),
                    scalar1=lnq_g_sb[:, s:s + 1],
                    scalar2=lnq_b_sb[:, s:s + 1],
                    op0=ALU.mult, op1=ALU.add,
                )

        # ============ phase K + attention (interleaved) ============
        with (
            tc.tile_pool(name="kp", bufs=1) as kp,
            tc.tile_pool(name="wrot2", bufs=1) as wrot2,
            tc.tile_pool(name="post", bufs=2) as postp,
            tc.tile_pool(name="stat2", bufs=6) as stat2,
            tc.tile_pool(name="attn", bufs=3) as apool,
            tc.tile_pool(name="hwork", bufs=4) as hwork,
        ):
            wk_sb = wrot2.tile([P, G6, D], bf16)
            nc.sync.dma_start(
                out=wk_sb,
                in_=wkvT.rearrange("(s p) o -> p s o", p=P)[:, :, 0:D],
            )
            knat_sb = kp.tile([P, MCH, D], bf16)

            kt_psum = ExitStack()
            ps_kt = kt_psum.enter_context(
                tc.tile_pool(name="ps_kt", bufs=2, space="PSUM")
            )

            def kt_stt(s, half, pt, posT_s):
                nc.vector.scalar_tensor_tensor(
                    out=kT_sb[:, s, half * 1024:(half + 1) * 1024],
                    in0=pt.rearrange("p c n -> p (c n)"),
                    scalar=lnk_g_sb[:, s:s + 1],
                    in1=posT_s[:, half * 1024:(half + 1) * 1024],
                    op0=ALU.mult, op1=ALU.add,
                )

            def build_kt_block(s):
                posT_s = postp.tile([P, N], bf16, tag="posT", name="posT_s")
                nc.sync.dma_start(
                    out=posT_s,
                    in_=posTb.rearrange("(s p) n -> p s n", p=P)[:, s, :],
                )
                for half in range(2):
                    pt = ps_kt.tile([P, MCH // 2, P], bf16, tag="pt", name="pt")
                    for c in range(MCH // 2):
                        mc = half * 8 + c
                        nc.tensor.transpose(
                            pt[:, c], knat_sb[:, mc, s * P:(s + 1) * P], ident_bf
                        )
                    kt_stt(s, half, pt, posT_s)

            with tc.tile_pool(name="ps_pk", bufs=3, space="PSUM") as ps_pk:
                for c in range(MCH):
                    py = ps_pk.tile([P, D], f32, tag="pyk")
                    for o0, o1 in HALVES:
                        for s in range(G6):
                            nc.tensor.matmul(
                                py[:, o0:o1],
                                xvT_sb[:, s, c * P:(c + 1) * P],
                                wk_sb[:, s, o0:o1],
                                start=(s == 0),
                                stop=(not has_qkv_bias and s == G6 - 1),
                            )
                        if has_qkv_bias:
                            nc.tensor.matmul(
                                py[:, o0:o1], ones_r, bqkv_sb[:, D + o0:D + o1],
                                start=False, stop=True,
                            )
                    negmr, rstd = ln_stats(py, stat2)
                    nc.scalar.activation(
                        out=knat_sb[:, c, :], in_=py, func=ACTF.Identity,
                        bias=negmr, scale=rstd,
                    )

            attn_psum = ExitStack()
            ps_s = attn_psum.enter_context(
                tc.tile_pool(name="ps_s", bufs=2, space="PSUM")
            )
            ps_o = attn_psum.enter_context(
                tc.tile_pool(name="ps_o", bufs=2, space="PSUM")
            )

            # preload the Exp act table while PE is still on K-proj: the
            # LN applies (Identity) work under any table, so this is the
            # only swap and it runs off the critical path
            dummy = hwork.tile([P, 1], bf16, tag="dummy", name="dummy")
            nc.scalar.activation(out=dummy, in_=eps_t, func=ACTF.Exp)

            def head(h):
                s, p0 = h // 2, DH * (h % 2)
                po = ps_o.tile([DH + 1, S], f32, tag="po", name="po")
                for mc0 in range(0, MCH, 2):
                    ps = ps_s.tile([P, 2, S], f32, tag="ps2", name="ps")
                    for j in range(2):
                        mc = mc0 + j
                        nc.tensor.matmul(
                            ps[:, j],
                            kT_sb[p0:p0 + DH, s, mc * P:(mc + 1) * P],
                            qT_sb[p0:p0 + DH, s, :],
                            start=True, stop=True,
                        )
                    at = apool.tile([P, 2, S], bf16, tag="at", name="at")
                    nc.scalar.activation(
                        out=at, in_=ps, func=ACTF.Exp, scale=SCALE
                    )
                    for j in range(2):
                        mc = mc0 + j
                        nc.tensor.matmul(
                            po, vaug_sb[:, mc, h, :], at[:, j],
                            start=(mc == 0), stop=(mc == MCH - 1),
                        )
                rinv = hwork.tile([1, S], f32, tag="rinv", name="rinv")
                nc.vector.reciprocal(out=rinv, in_=po[DH:DH + 1, :])
                rbc = hwork.tile([DH, S], f32, tag="rbc", name="rbc")
                nc.gpsimd.partition_broadcast(rbc, rinv)
                nc.vector.tensor_tensor(
                    out=outT_sb[p0:p0 + DH, s, :], in0=po[0:DH, :],
                    in1=rbc, op=ALU.mult,
                )

            # software-pipelined: kT block s+1 is built between the two heads
            # of block s so its transposes/stt hide under the exp-bound heads
            build_kt_block(0)
            for s in range(G6):
                head(2 * s)
                if s + 1 < G6:
                    build_kt_block(s + 1)
                head(2 * s + 1)
            attn_psum.close()
            kt_psum.close()

        # ========== phase Z: out-proj, gate, fuse, final LN ==========
        with (
            tc.tile_pool(name="zw", bufs=1) as zw,
            tc.tile_pool(name="zs", bufs=2) as zs,
            tc.tile_pool(name="stat3", bufs=4) as stat3,
            tc.tile_pool(name="ps_z", bufs=2, space="PSUM") as ps_z,
            tc.tile_pool(name="ps_g", bufs=2, space="PSUM") as ps_g,
        ):
            woT_sb = zw.tile([P, G6, D], bf16)
            nc.sync.dma_start(out=woT_sb, in_=woT.rearrange("(s p) o -> p s o", p=P))
            gwvT_sb = zw.tile([P, G6, D], bf16)
            nc.scalar.dma_start(
                out=gwvT_sb, in_=gwvT.rearrange("(s p) o -> p s o", p=P)
            )
            gwzT_sb = zw.tile([P, G6, D], bf16)
            nc.gpsimd.dma_start(
                out=gwzT_sb, in_=gwzT.rearrange("(s p) o -> p s o", p=P)
            )
            bo_sb = gb_sb = None
            if has_o_bias:
                bo_sb = zw.tile([1, D], bf16)
                nc.sync.dma_start(out=bo_sb, in_=bo_a.ap())
            if has_g_bias:
                gb_sb = zw.tile([1, D], bf16)
                nc.sync.dma_start(out=gb_sb, in_=gb_e.ap())
            gbc = bbc = None
            if has_f_affine:
                gbc = zw.tile([P, D], f32)
                bbc = zw.tile([P, D], f32)
                for dst, src_row in ((gbc, lnfw_sb), (bbc, lnfb_sb)):
                    pb = ps_z.tile([P, D], f32, tag="pz")
                    for o0, o1 in HALVES:
                        nc.tensor.matmul(
                            pb[:, o0:o1], ones_r_f, src_row[:, o0:o1],
                            start=True, stop=True,
                        )
                    nc.vector.tensor_copy(out=dst, in_=pb)

            # loop A (Sigmoid/Copy table): gate + z + fuse per chunk
            fusses = []
            for c in range(NCH):
                vis_c = zs.tile([P, D], f32, tag="vis", name="vis_c")
                nc.sync.dma_start(
                    out=vis_c, in_=vis_nat.rearrange("(c p) o -> p c o", p=P)[:, c]
                )

                # gate: vis-half (own xv columns are [0, 512)) + folded z-half
                pg = ps_g.tile([P, D], f32, tag="pg")
                for o0, o1 in HALVES:
                    for s in range(G6):
                        nc.tensor.matmul(
                            pg[:, o0:o1],
                            xvT_sb[:, s, c * P:(c + 1) * P],
                            gwvT_sb[:, s, o0:o1],
                            start=(s == 0), stop=False,
                        )
                    for s in range(G6):
                        last = (not has_g_bias) and s == G6 - 1
                        nc.tensor.matmul(
                            pg[:, o0:o1],
                            outT_sb[:, s, c * P:(c + 1) * P],
                            gwzT_sb[:, s, o0:o1],
                            start=False, stop=last,
                        )
                    if has_g_bias:
                        nc.tensor.matmul(
                            pg[:, o0:o1], ones_b, gb_sb[:, o0:o1],
                            start=False, stop=True,
                        )
                gsig = zs.tile([P, D], bf16, tag="gsig", name="gsig")
                nc.scalar.activation(out=gsig, in_=pg, func=ACTF.Sigmoid)

                # out-proj z
                pz = ps_z.tile([P, D], f32, tag="pz")
                for o0, o1 in HALVES:
                    for s in range(G6):
                        last = (not has_o_bias) and s == G6 - 1
                        nc.tensor.matmul(
                            pz[:, o0:o1],
                            outT_sb[:, s, c * P:(c + 1) * P],
                            woT_sb[:, s, o0:o1],
                            start=(s == 0), stop=last,
                        )
                    if has_o_bias:
                        nc.tensor.matmul(
                            pz[:, o0:o1], ones_b, bo_sb[:, o0:o1],
                            start=False, stop=True,
                        )
                z_c = zs.tile([P, D], f32, tag="zc", name="z_c")
                nc.scalar.copy(out=z_c, in_=pz)

                # fuse: z + g*(vis - z)
                dvz = zs.tile([P, D], f32, tag="dvz", name="dvz")
                nc.gpsimd.tensor_tensor(out=dvz, in0=vis_c, in1=z_c, op=ALU.subtract)
                fus = zs.tile([P, D], f32, tag=f"fus{c}", name="fus")
                nc.vector.tensor_tensor(out=fus, in0=gsig, in1=dvz, op=ALU.mult)
                nc.vector.tensor_tensor(out=fus, in0=fus, in1=z_c, op=ALU.add)
                fusses.append(fus)

            # loop B (Sqrt/Identity table): final LayerNorm + store
            for c in range(NCH):
                fus = fusses[c]
                negmr, rstd = ln_stats(fus, stat3)
                tnorm = zs.tile([P, D], f32, tag="tnorm", name="tnorm")
                nc.scalar.activation(
                    out=tnorm, in_=fus, func=ACTF.Identity, bias=negmr, scale=rstd
                )
                if has_f_affine:
                    nc.vector.tensor_tensor(out=tnorm, in0=tnorm, in1=gbc, op=ALU.mult)
                    nc.vector.tensor_tensor(out=tnorm, in0=tnorm, in1=bbc, op=ALU.add)
                nc.sync.dma_start(
                    out=out_rows.rearrange("(c p) o -> p c o", p=P)[:, c], in_=tnorm
                )

    nc.compile()
    return nc


def _prepare_in_maps(inputs):
    f32 = np.float32
    vis = np.asarray(inputs["visible_features"], f32)
    inf = np.asarray(inputs["infrared_features"], f32)
    wq = np.asarray(inputs["wq"], f32)
    bq = np.asarray(inputs["bq"], f32)
    lnq_w = np.asarray(inputs["lnq_w"], f32)
    lnq_b = np.asarray(inputs["lnq_b"], f32)
    wk = np.asarray(inputs["wk"], f32)
    bk = np.asarray(inputs["bk"], f32)
    lnk_w = np.asarray(inputs["lnk_w"], f32)
    lnk_b = np.asarray(inputs["lnk_b"], f32)
    wv = np.asarray(inputs["wv"], f32)
    bv = np.asarray(inputs["bv"], f32)
    lnv_w = np.asarray(inputs["lnv_w"], f32)
    lnv_b = np.asarray(inputs["lnv_b"], f32)
    pos = np.asarray(inputs["pos_emb"], f32)[:N]
    wo = np.asarray(inputs["wo"], f32)
    bo = np.asarray(inputs["bo"], f32)
    gw = np.asarray(inputs["gate_w"], f32)
    gb_ = np.asarray(inputs["gate_b"], f32)
    ln_w = np.asarray(inputs["ln_w"], f32)
    ln_b = np.asarray(inputs["ln_b"], f32)

    wo_eff = wo * lnv_w[None, :]
    bo_a = bo + wo @ lnv_b
    gwv = gw[:, :D]
    gwz = gw[:, D:]
    W_eff = (gwz.astype(np.float64) @ wo_eff.astype(np.float64)).astype(f32)
    gb_eff = gb_ + gwz @ bo_a

    wqT = np.ascontiguousarray(wq.T)
    wkvT = np.ascontiguousarray(np.concatenate([wk.T, wv.T], axis=1)).astype(BF)
    woTb = np.ascontiguousarray(wo_eff.T).astype(BF)
    gwvTb = np.ascontiguousarray(gwv.T).astype(BF)
    gwzTb = np.ascontiguousarray(W_eff.T).astype(BF)
    bqkv = np.ascontiguousarray(np.concatenate([bq, bk, bv])[None])
    bo_ab = np.ascontiguousarray(bo_a[None]).astype(BF)
    gb_eb = np.ascontiguousarray(gb_eff[None]).astype(BF)
    lnq_g2 = np.ascontiguousarray(lnq_w.reshape(G6, P).T)
    lnq_b2 = np.ascontiguousarray(lnq_b.reshape(G6, P).T)
    lnk_g2 = np.ascontiguousarray(lnk_w.reshape(G6, P).T)
    lnf = np.stack([ln_w, ln_b])

    flags = (
        bool(np.any(bq) or np.any(bk) or np.any(bv)),
        bool(np.any(bo_a)),
        bool(np.any(gb_eff)),
        bool(np.any(ln_b) or np.any(ln_w != 1.0)),
    )

    posT_base = pos.T / SCALE + lnk_b[:, None]   # [D, N]

    in_maps = []
    for c in range(CORES):
        b, r0 = c // GROUP, (c % GROUP) * S
        perm = np.concatenate(
            [np.arange(r0, r0 + S), np.arange(0, r0), np.arange(r0 + S, N)]
        )
        in_maps.append({
            "xqT": np.ascontiguousarray(inf[b, r0:r0 + S].T),
            "xvT": np.ascontiguousarray(vis[b][perm].T).astype(BF),
            "vis_nat": np.ascontiguousarray(vis[b, r0:r0 + S]),
            "posTb": np.ascontiguousarray(posT_base[:, perm]).astype(BF),
            "wqT": wqT,
            "wkvT": wkvT,
            "woT": woTb,
            "gwvT": gwvTb,
            "gwzT": gwzTb,
            "lnq_g": lnq_g2,
            "lnq_b": lnq_b2,
            "lnk_g": lnk_g2,
            "bqkv": bqkv,
            "bo_a": bo_ab,
            "gb_e": gb_eb,
            "lnf": lnf,
        })
    return in_maps, flags


def kernel(trace=False, **inputs):
    from concourse.bass_utils import run_bass_kernel_spmd

    in_maps, flags = _prepare_in_maps(inputs)
    key = ("nc",) + flags
    if key not in _CACHE:
        _CACHE[key] = _build(*flags)
    nc = _CACHE[key]
    res = run_bass_kernel_spmd(
        nc, in_maps, core_ids=list(range(CORES)), trace=trace
    )
    out = np.empty((B, N, D), np.float32)
    for c in range(CORES):
        b, r0 = c // GROUP, (c % GROUP) * S
        out[b, r0:r0 + S] = res.results[c]["out_rows"]
    _CACHE["last_result"] = res
    _CACHE["nc"] = nc
    return out
